# revision 27
# baseline (speedup 1.0000x reference)
"""Trainium2 Bass kernel for nn_CoAttention (gnn_message_passing).

Math (per batch b, derived from the reference):
    h1 = x1 @ Wk.T ; h2 = x2 @ Wk.T
    L  = h1 @ h2.T                                  # [N, M] logits
    E  = exp(L - C)                                 # any constant C cancels
    r  = E.sum(axis=1)  (row sums over m)
    c  = E.sum(axis=0)  (col sums over n)
    softmax(dim=1) quirk of the reference means:
    n12 = E @ (v2 / c[:, None])                     # v2 = x2 @ Wv.T
    n21 = E.T @ (v1 / r[:, None])                   # v1 = x1 @ Wv.T
    msg1 = leaky_relu(n12 @ Wo.T + bo, 0.01)
    msg2 = leaky_relu(n21 @ Wo.T + bo, 0.01)

Sharding: data-parallel over batch (8 batches -> 8 cores), weights replicated.

On-core strategy: E (16 MB fp32) is never materialized in full.  Both uses
of E contract over a different axis, and each pass streams 128-row tiles of
E (or E.T) through SBUF while PSUM accumulates the contraction:
  pass 1: for each n-tile i: E_i = exp(h1T[:, i].T @ h2T - C)  (ACT fused
          row-sum via accum_out) ; u1_i = v1_i * (1/r_i) ;
          acc21[k, m] += u1_i.T-contract:  matmul(lhsT=u1_i, rhs=E_i)
  pass 2: identical with roles swapped -> acc12, using col sums c_j.
The big matmuls run as float32r (full PE rate at free-dim 512); the value /
output projections stay fp32 (precision on the direct output path).
"""

import sys

if "/opt/trn_rl_repo" not in sys.path:
    sys.path.insert(0, "/opt/trn_rl_repo")

import numpy as np

import concourse.bass as bass
import concourse.mybir as mybir
import concourse.tile as tile
from concourse import bacc

P = 128
N = 2048
T = N // P          # 16 tiles
CH = 512            # matmul moving-dim chunk
NCH = N // CH       # 4 chunks
HALF = 1024         # logit psum tile free size (2 banks)
C_SHIFT = 100.0     # exp(L - C): global logit max is ~168, min axis-max ~46
F32 = mybir.dt.float32
F32R = mybir.dt.float32r

# dtype knobs for the matmul families.  float32r inputs must be PRODUCED
# as float32r (the BIR verifier enforces rounded producers), so the tiles
# feeding the big matmuls are allocated as F32R and written by DVE/ACT.
R_BIG = True        # projection/msg matmuls in float32r
BIG_DT = F32R if R_BIG else F32
# logit + accumulation matmuls run in bf16 (1 cyc/row vs f32r's 2);
# set to F32R to trade ~2x PE time on the big matmuls for ~8x less
# rounding error on the attention path.
E_DT = mybir.dt.bfloat16


def build_bass():
    nc = bacc.Bacc("TRN2", target_bir_lowering=False)

    x1t_d = nc.dram_tensor("x1t", [P, N], BIG_DT, kind="ExternalInput")
    x2t_d = nc.dram_tensor("x2t", [P, N], BIG_DT, kind="ExternalInput")
    wkT_d = nc.dram_tensor("wkT", [P, P], BIG_DT, kind="ExternalInput")
    wvT_d = nc.dram_tensor("wvT", [P, P], BIG_DT, kind="ExternalInput")
    woT_d = nc.dram_tensor("woT", [P, P], BIG_DT, kind="ExternalInput")
    bo_d = nc.dram_tensor("bo", [P, 1], F32, kind="ExternalInput")
    # outputs are stored transposed ([out_feature, node]); the host
    # wrapper transposes back during the gather
    msg1_d = nc.dram_tensor("msg1", [P, N], F32, kind="ExternalOutput")
    msg2_d = nc.dram_tensor("msg2", [P, N], F32, kind="ExternalOutput")

    with tile.TileContext(nc) as tc:
        with (
            tc.tile_pool(name="const", bufs=1) as cpool,
            tc.tile_pool(name="big", bufs=1) as big,
            tc.tile_pool(name="work", bufs=3) as work,
            tc.tile_pool(name="lp", bufs=2, space="PSUM") as lp,
            tc.tile_pool(name="accp", bufs=1, space="PSUM") as accp,
        ):
            # ---- loads ----
            wkT = cpool.tile([P, P], BIG_DT, tag="wkT")
            nc.sync.dma_start(wkT[:], wkT_d[:])
            wvT = cpool.tile([P, P], BIG_DT, tag="wvT")
            nc.sync.dma_start(wvT[:], wvT_d[:])
            woT = cpool.tile([P, P], BIG_DT, tag="woT")
            nc.sync.dma_start(woT[:], woT_d[:])
            bo_t = cpool.tile([P, 1], F32, tag="bo")
            nc.sync.dma_start(bo_t[:], bo_d[:])
            negc = cpool.tile([P, 1], F32, tag="negc")
            nc.vector.memset(negc[:], -C_SHIFT)

            x1t = big.tile([P, N], BIG_DT, tag="x1t")
            x2t = big.tile([P, N], BIG_DT, tag="x2t")
            # split the 1MB loads into chunks so compute can start early
            for ch in range(NCH):
                s = slice(ch * CH, (ch + 1) * CH)
                nc.sync.dma_start(x1t[:, s], x1t_d[:, s])
                nc.sync.dma_start(x2t[:, s], x2t_d[:, s])

            # ---- h projections: hXt[k, n] (fp32 matmul; copy rounds to
            # float32r so the logit matmuls can run at full PE rate) ----
            h1t = big.tile([P, N], E_DT, tag="h1t")
            h2t = big.tile([P, N], E_DT, tag="h2t")
            for dst, src in ((h1t, x1t), (h2t, x2t)):
                for half in range(N // HALF):
                    ps = lp.tile([P, HALF], F32, tag="lp")
                    for q in range(HALF // CH):
                        lo = half * HALF + q * CH
                        nc.tensor.matmul(
                            ps[:, q * CH:(q + 1) * CH],
                            wkT[:],
                            src[:, lo:lo + CH],
                        )
                    nc.vector.tensor_copy(
                        dst[:, half * HALF:(half + 1) * HALF], ps[:]
                    )

            # ---- v projections: natural tiles v[n_sub, k] (f32r) ----
            v1s = big.tile([P, T, P], F32, tag="v1s")
            v2s = big.tile([P, T, P], F32, tag="v2s")

            def emit_v(dst, src):
                for i in range(T):
                    ps = lp.tile([P, HALF], F32, tag="lp")
                    nc.tensor.matmul(
                        ps[:, :P], src[:, i * P:(i + 1) * P], wvT[:]
                    )
                    nc.vector.tensor_copy(dst[:, i, :], ps[:, :P])

            emit_v(v1s, x1t)

            # ---- the two streaming passes ----
            def stream_pass(ha, hb, vsrc, acc_tag):
                """Accumulate accT[k, m] = sum_n u[n, k] * exp(ha_n . hb_m - C)
                where u = vsrc / rowsum.  Returns the SBUF copy of the
                accumulated [P, N] tensor."""
                acc = accp.tile([P, N], F32, tag="acc")
                for i in range(T):
                    ei = work.tile([P, N], E_DT, tag="E")
                    for half in range(N // HALF):
                        ps = lp.tile([P, HALF], F32, tag="lp")
                        for q in range(HALF // CH):
                            lo = half * HALF + q * CH
                            nc.tensor.matmul(
                                ps[:, q * CH:(q + 1) * CH],
                                ha[:, i * P:(i + 1) * P],
                                hb[:, lo:lo + CH],
                            )
                        nc.scalar.activation(
                            ei[:, half * HALF:(half + 1) * HALF],
                            ps[:],
                            mybir.ActivationFunctionType.Exp,
                            bias=negc[:],
                        )
                    # row sums on DVE: in-place multiply by 1.0 with
                    # accum_out (bf16 4x mode; frees ACT of the
                    # accumulate/read tax)
                    rsum = work.tile([P, 1], F32, tag="rsum")
                    nc.vector.tensor_scalar(
                        ei[:],
                        ei[:],
                        1.0,
                        0.0,
                        mybir.AluOpType.mult,
                        mybir.AluOpType.add,
                        accum_out=rsum[:],
                    )
                    rrec = work.tile([P, 1], F32, tag="rrec")
                    nc.vector.reciprocal(rrec[:], rsum[:])
                    u = work.tile([P, P], E_DT, tag="u")
                    nc.vector.tensor_scalar_mul(u[:], vsrc[:, i, :], rrec[:])
                    for ch in range(NCH):
                        nc.tensor.matmul(
                            acc[:, ch * CH:(ch + 1) * CH],
                            u[:],
                            ei[:, ch * CH:(ch + 1) * CH],
                            start=(i == 0),
                            stop=(i == T - 1),
                        )
                accs = big.tile([P, N], BIG_DT, tag=acc_tag)
                nc.vector.tensor_copy(accs[:], acc[:])
                return accs

            n21t = stream_pass(h1t, h2t, v1s, "n21t")   # [k, m]
            emit_v(v2s, x2t)
            n12t = stream_pass(h2t, h1t, v2s, "n12t")   # [k, n]

            # ---- output projections + bias + leaky relu + transpose ----
            def emit_msg(nt, out_d):
                ps = accp.tile([P, N], F32, tag="acc")
                for ch in range(NCH):
                    s = slice(ch * CH, (ch + 1) * CH)
                    nc.tensor.matmul(ps[:, s], woT[:], nt[:, s])
                msgT = work.tile([P, N], F32, tag="msgT")
                nc.scalar.activation(
                    msgT[:],
                    ps[:],
                    mybir.ActivationFunctionType.Lrelu,
                    bias=bo_t[:],
                    scale=1.0,
                    alpha=0.01,
                )
                nc.sync.dma_start(out_d[:], msgT[:])

            emit_msg(n21t, msg2_d)
            emit_msg(n12t, msg1_d)

    nc.compile()
    return nc


_NC_CACHE = None


def _get_nc():
    global _NC_CACHE
    if _NC_CACHE is None:
        _NC_CACHE = build_bass()
    return _NC_CACHE


def _make_in_maps(x1, x2, Wk, Wv, Wo, bo):
    x1 = np.ascontiguousarray(x1, dtype=np.float32)
    x2 = np.ascontiguousarray(x2, dtype=np.float32)
    wkT = np.ascontiguousarray(np.asarray(Wk, dtype=np.float32).T)
    wvT = np.ascontiguousarray(np.asarray(Wv, dtype=np.float32).T)
    woT = np.ascontiguousarray(np.asarray(Wo, dtype=np.float32).T)
    boc = np.ascontiguousarray(
        np.asarray(bo, dtype=np.float32).reshape(P, 1)
    )
    in_maps = []
    for b in range(x1.shape[0]):
        in_maps.append(
            {
                "x1t": np.ascontiguousarray(x1[b].T),
                "x2t": np.ascontiguousarray(x2[b].T),
                "wkT": wkT,
                "wvT": wvT,
                "woT": woT,
                "bo": boc,
            }
        )
    return in_maps


def run(x1, x2, Wk, Wv, Wo, bo, trace=False, tmpdir=None):
    from concourse import bass_utils

    nc = _get_nc()
    in_maps = _make_in_maps(x1, x2, Wk, Wv, Wo, bo)
    res = bass_utils.run_bass_kernel_spmd(
        nc, in_maps, core_ids=list(range(len(in_maps))), trace=trace,
        tmpdir=tmpdir,
    )
    msg1 = np.stack([np.ascontiguousarray(r["msg1"].T) for r in res.results])
    msg2 = np.stack([np.ascontiguousarray(r["msg2"].T) for r in res.results])
    return (msg1, msg2), res


def kernel(x1, x2, Wk, Wv, Wo, bo):
    out, _ = run(x1, x2, Wk, Wv, Wo, bo, trace=False)
    return out


# revision 29
# speedup vs baseline: 1.2063x; 1.2063x over previous
"""Trainium2 Bass kernel for nn_CoAttention (gnn_message_passing).

Math (per batch b, derived from the reference):
    h1 = x1 @ Wk.T ; h2 = x2 @ Wk.T
    L  = h1 @ h2.T                                  # [N, M] logits
    E  = exp(L - C)                                 # any constant C cancels
    r  = E.sum(axis=1)  (row sums over m)
    c  = E.sum(axis=0)  (col sums over n)
    softmax(dim=1) quirk of the reference means:
    n12 = E @ (v2 / c[:, None])                     # v2 = x2 @ Wv.T
    n21 = E.T @ (v1 / r[:, None])                   # v1 = x1 @ Wv.T
    msg1 = leaky_relu(n12 @ Wo.T + bo, 0.01)
    msg2 = leaky_relu(n21 @ Wo.T + bo, 0.01)

Sharding: data-parallel over batch (8 batches -> 8 cores), weights replicated.

On-core strategy: E (16 MB fp32) is never materialized in full.  Both uses
of E contract over a different axis, and each pass streams 128-row tiles of
E (or E.T) through SBUF while PSUM accumulates the contraction:
  pass 1: for each n-tile i: E_i = exp(h1T[:, i].T @ h2T - C)  (ACT fused
          row-sum via accum_out) ; u1_i = v1_i * (1/r_i) ;
          acc21[k, m] += u1_i.T-contract:  matmul(lhsT=u1_i, rhs=E_i)
  pass 2: identical with roles swapped -> acc12, using col sums c_j.
The big matmuls run as float32r (full PE rate at free-dim 512); the value /
output projections stay fp32 (precision on the direct output path).
"""

import sys

if "/opt/trn_rl_repo" not in sys.path:
    sys.path.insert(0, "/opt/trn_rl_repo")

import numpy as np

import concourse.bass as bass
import concourse.mybir as mybir
import concourse.tile as tile
from concourse import bacc

P = 128
N = 2048
T = N // P          # 16 tiles
CH = 512            # matmul moving-dim chunk
NCH = N // CH       # 4 chunks
HALF = 1024         # logit psum tile free size (2 banks)
C_SHIFT = 100.0     # exp(L - C): global logit max is ~168, min axis-max ~46
F32 = mybir.dt.float32
F32R = mybir.dt.float32r

# dtype knobs for the matmul families.  float32r inputs must be PRODUCED
# as float32r (the BIR verifier enforces rounded producers), so the tiles
# feeding the big matmuls are allocated as F32R and written by DVE/ACT.
R_BIG = True        # projection/msg matmuls in float32r
BIG_DT = F32R if R_BIG else F32
# logit + accumulation matmuls run in bf16 (1 cyc/row vs f32r's 2);
# set to F32R to trade ~2x PE time on the big matmuls for ~8x less
# rounding error on the attention path.
E_DT = mybir.dt.bfloat16


def build_bass():
    nc = bacc.Bacc("TRN2", target_bir_lowering=False)

    x1t_d = nc.dram_tensor("x1t", [P, N], BIG_DT, kind="ExternalInput")
    x2t_d = nc.dram_tensor("x2t", [P, N], BIG_DT, kind="ExternalInput")
    wkT_d = nc.dram_tensor("wkT", [P, P], BIG_DT, kind="ExternalInput")
    wvT_d = nc.dram_tensor("wvT", [P, P], BIG_DT, kind="ExternalInput")
    woT_d = nc.dram_tensor("woT", [P, P], BIG_DT, kind="ExternalInput")
    bo_d = nc.dram_tensor("bo", [P, 1], F32, kind="ExternalInput")
    # outputs are stored transposed ([out_feature, node]); the host
    # wrapper transposes back during the gather
    msg1_d = nc.dram_tensor("msg1", [P, N], F32, kind="ExternalOutput")
    msg2_d = nc.dram_tensor("msg2", [P, N], F32, kind="ExternalOutput")

    with tile.TileContext(nc) as tc:
        with (
            tc.tile_pool(name="const", bufs=1) as cpool,
            tc.tile_pool(name="big", bufs=1) as big,
            tc.tile_pool(name="work", bufs=3) as work,
            tc.tile_pool(name="lp", bufs=2, space="PSUM") as lp,
            tc.tile_pool(name="accp", bufs=1, space="PSUM") as accp,
        ):
            # ---- loads ----
            wkT = cpool.tile([P, P], BIG_DT, tag="wkT")
            nc.sync.dma_start(wkT[:], wkT_d[:])
            wvT = cpool.tile([P, P], BIG_DT, tag="wvT")
            nc.sync.dma_start(wvT[:], wvT_d[:])
            woT = cpool.tile([P, P], BIG_DT, tag="woT")
            nc.sync.dma_start(woT[:], woT_d[:])
            bo_t = cpool.tile([P, 1], F32, tag="bo")
            nc.sync.dma_start(bo_t[:], bo_d[:])
            negc = cpool.tile([P, 1], F32, tag="negc")
            nc.vector.memset(negc[:], -C_SHIFT)

            # ---- PE warmup: ~5us of dummy matmuls with no DMA deps so the
            # HAM clock-gate reaches 8/8 before the real work arrives ----
            wupa = cpool.tile([P, CH], E_DT, tag="wupa")
            nc.vector.memset(wupa[:], 0.5)
            for w in range(20):
                wps = lp.tile([P, HALF], F32, tag="lp")
                nc.tensor.matmul(wps[:, :CH], wupa[:, :P], wupa[:])

            x1t = big.tile([P, N], BIG_DT, tag="x1t")
            x2t = big.tile([P, N], BIG_DT, tag="x2t")
            # split the 1MB loads into chunks so compute can start early
            for ch in range(NCH):
                s = slice(ch * CH, (ch + 1) * CH)
                nc.sync.dma_start(x1t[:, s], x1t_d[:, s])
                nc.sync.dma_start(x2t[:, s], x2t_d[:, s])

            # ---- h projections: hXt[k, n] (fp32 matmul; copy rounds to
            # float32r so the logit matmuls can run at full PE rate) ----
            h1t = big.tile([P, N], E_DT, tag="h1t")
            h2t = big.tile([P, N], E_DT, tag="h2t")
            for dst, src in ((h1t, x1t), (h2t, x2t)):
                for half in range(N // HALF):
                    ps = lp.tile([P, HALF], F32, tag="lp")
                    for q in range(HALF // CH):
                        lo = half * HALF + q * CH
                        nc.tensor.matmul(
                            ps[:, q * CH:(q + 1) * CH],
                            wkT[:],
                            src[:, lo:lo + CH],
                        )
                    nc.vector.tensor_copy(
                        dst[:, half * HALF:(half + 1) * HALF], ps[:]
                    )

            # ---- v projections: natural tiles v[n_sub, k] (f32r) ----
            v1s = big.tile([P, T, P], F32, tag="v1s")
            v2s = big.tile([P, T, P], F32, tag="v2s")

            def emit_v(dst, src):
                for i in range(T):
                    ps = lp.tile([P, HALF], F32, tag="lp")
                    nc.tensor.matmul(
                        ps[:, :P], src[:, i * P:(i + 1) * P], wvT[:]
                    )
                    nc.vector.tensor_copy(dst[:, i, :], ps[:, :P])

            emit_v(v1s, x1t)

            # ---- the two streaming passes ----
            def stream_pass(ha, hb, vsrc, acc_tag):
                """Accumulate accT[k, m] = sum_n u[n, k] * exp(ha_n . hb_m - C)
                where u = vsrc / rowsum.  Returns the SBUF copy of the
                accumulated [P, N] tensor."""
                acc = accp.tile([P, N], F32, tag="acc")
                for i in range(T):
                    ei = work.tile([P, N], E_DT, tag="E")
                    racc = work.tile([P, 2], F32, tag="racc")
                    for half in range(N // HALF):
                        ps = lp.tile([P, HALF], F32, tag="lp")
                        for q in range(HALF // CH):
                            lo = half * HALF + q * CH
                            nc.tensor.matmul(
                                ps[:, q * CH:(q + 1) * CH],
                                ha[:, i * P:(i + 1) * P],
                                hb[:, lo:lo + CH],
                            )
                        nc.scalar.activation(
                            ei[:, half * HALF:(half + 1) * HALF],
                            ps[:],
                            mybir.ActivationFunctionType.Exp,
                            bias=negc[:],
                            accum_out=racc[:, half:half + 1],
                        )
                    rsum = work.tile([P, 1], F32, tag="rsum")
                    nc.vector.tensor_add(rsum[:], racc[:, 0:1], racc[:, 1:2])
                    rrec = work.tile([P, 1], F32, tag="rrec")
                    nc.vector.reciprocal(rrec[:], rsum[:])
                    u = work.tile([P, P], E_DT, tag="u")
                    nc.vector.tensor_scalar_mul(u[:], vsrc[:, i, :], rrec[:])
                    for ch in range(NCH):
                        nc.tensor.matmul(
                            acc[:, ch * CH:(ch + 1) * CH],
                            u[:],
                            ei[:, ch * CH:(ch + 1) * CH],
                            start=(i == 0),
                            stop=(i == T - 1),
                        )
                accs = big.tile([P, N], BIG_DT, tag=acc_tag)
                nc.vector.tensor_copy(accs[:], acc[:])
                return accs

            n21t = stream_pass(h1t, h2t, v1s, "n21t")   # [k, m]
            emit_v(v2s, x2t)
            n12t = stream_pass(h2t, h1t, v2s, "n12t")   # [k, n]

            # ---- output projections + bias + leaky relu + transpose ----
            def emit_msg(nt, out_d):
                ps = accp.tile([P, N], F32, tag="acc")
                for ch in range(NCH):
                    s = slice(ch * CH, (ch + 1) * CH)
                    nc.tensor.matmul(ps[:, s], woT[:], nt[:, s])
                msgT = work.tile([P, N], F32, tag="msgT")
                nc.scalar.activation(
                    msgT[:],
                    ps[:],
                    mybir.ActivationFunctionType.Lrelu,
                    bias=bo_t[:],
                    scale=1.0,
                    alpha=0.01,
                )
                nc.sync.dma_start(out_d[:], msgT[:])

            emit_msg(n21t, msg2_d)
            emit_msg(n12t, msg1_d)

    nc.compile()
    return nc


_NC_CACHE = None


def _get_nc():
    global _NC_CACHE
    if _NC_CACHE is None:
        _NC_CACHE = build_bass()
    return _NC_CACHE


def _make_in_maps(x1, x2, Wk, Wv, Wo, bo):
    x1 = np.ascontiguousarray(x1, dtype=np.float32)
    x2 = np.ascontiguousarray(x2, dtype=np.float32)
    wkT = np.ascontiguousarray(np.asarray(Wk, dtype=np.float32).T)
    wvT = np.ascontiguousarray(np.asarray(Wv, dtype=np.float32).T)
    woT = np.ascontiguousarray(np.asarray(Wo, dtype=np.float32).T)
    boc = np.ascontiguousarray(
        np.asarray(bo, dtype=np.float32).reshape(P, 1)
    )
    in_maps = []
    for b in range(x1.shape[0]):
        in_maps.append(
            {
                "x1t": np.ascontiguousarray(x1[b].T),
                "x2t": np.ascontiguousarray(x2[b].T),
                "wkT": wkT,
                "wvT": wvT,
                "woT": woT,
                "bo": boc,
            }
        )
    return in_maps


def run(x1, x2, Wk, Wv, Wo, bo, trace=False, tmpdir=None):
    from concourse import bass_utils

    nc = _get_nc()
    in_maps = _make_in_maps(x1, x2, Wk, Wv, Wo, bo)
    res = bass_utils.run_bass_kernel_spmd(
        nc, in_maps, core_ids=list(range(len(in_maps))), trace=trace,
        tmpdir=tmpdir,
    )
    msg1 = np.stack([np.ascontiguousarray(r["msg1"].T) for r in res.results])
    msg2 = np.stack([np.ascontiguousarray(r["msg2"].T) for r in res.results])
    return (msg1, msg2), res


def kernel(x1, x2, Wk, Wv, Wo, bo):
    out, _ = run(x1, x2, Wk, Wv, Wo, bo, trace=False)
    return out
